# revision 17
# baseline (speedup 1.0000x reference)
"""Trainium2 Bass kernel for autoregressive GRU sampling.

Problem: B=16384 samples, 1024 sequential sites, hidden=64, PyTorch GRU-cell
math with gates [r,z,n], Bernoulli sampling via pre-drawn uniforms.

Strategy:
  - Pure data parallel over 8 cores (2048 samples/core); the 1024-site scan
    is local to each core.
  - Packed layout: the core's 2048 samples split into halves A/B; partitions
    carry hidden(64) x half(2), free dim carries 1024 samples. Block-diagonal
    stationaries [128,128] contract both halves in one matmul group, so every
    elementwise op runs [128, 1024] instead of [64, 2048] (DVE/ACT cost is
    proportional to free size, halving the elementwise chain).
  - Gate biases ride the matmuls: the bits tile has a const-1.0 row, and the
    Q* stationaries carry [w_ih | bias] rows, so pre-activations arrive
    complete in PSUM. b_hn rides the rg STT scalar port; b_in rides qn.
  - The n-gate pre-activation is assembled IN PSUM: DVE writes
    rg = (ghn + b_hn) * r into a psum bank, then a K=3 matmul accumulates
    bit*w_ihn + b_in on top (start=False), so tanh reads the finished value
    and one DVE pass disappears.
  - fp32r matmuls (1 cycle/row vs fp32's 4): all matmul inputs are produced
    as float32r (DVE-rounded) per the BIR verifier's contract.
  - Host precomputes Uhat = logit(u) - head_b; the device compares
    Uhat < head_w.h' in logit space (is_lt), removing the head sigmoid.
  - Two 512-wide streams per site keep PE/ACT/DVE/Pool pipelined; d and zd
    run on GpSimd (SBUF-only ops) to offload DVE.
  - A 4-byte dummy LDWEIGHTS reading tanh output advances PE's observed-ACT
    tick so psum-WAR waits stay within Matmult's 2-sync-wait budget.
"""

import numpy as np
from contextlib import ExitStack

HIDDEN = 64
N_SITES = 1024
BATCH = 16384
N_CORES = 8
B_LOCAL = BATCH // N_CORES  # 2048
HB = B_LOCAL // 2           # 1024 samples per half
CH = 512                    # moving-operand chunk (one stream)
NS = HB // CH               # 2 streams

_BUILD_CACHE = {}


def _build(n_sites: int, compile: bool = True):
    import concourse.bass as bass
    import concourse.bacc as bacc
    import concourse.tile as tile
    from concourse import mybir

    f32 = mybir.dt.float32
    f32r = mybir.dt.float32r
    bf16 = mybir.dt.bfloat16
    AF = mybir.ActivationFunctionType
    OP = mybir.AluOpType
    J = max(1, (n_sites + 127) // 128)

    nc = bacc.Bacc()
    uhat_d = nc.dram_tensor("uhat", [128, J, 2, HB], f32, kind="ExternalInput")
    wz_d = nc.dram_tensor("wz", [128, 128], f32, kind="ExternalInput")
    wr_d = nc.dram_tensor("wr", [128, 128], f32, kind="ExternalInput")
    wn_d = nc.dram_tensor("wn", [128, 128], f32, kind="ExternalInput")
    qz_d = nc.dram_tensor("qz", [2, 128], f32, kind="ExternalInput")
    qr_d = nc.dram_tensor("qr", [2, 128], f32, kind="ExternalInput")
    qn_d = nc.dram_tensor("qn", [2, 128], f32, kind="ExternalInput")
    whd_d = nc.dram_tensor("whd", [128, 2], f32, kind="ExternalInput")
    bhn_d = nc.dram_tensor("bhn", [128, 1], f32, kind="ExternalInput")
    bz_d = nc.dram_tensor("bz", [128, 1], f32, kind="ExternalInput")
    br_d = nc.dram_tensor("br", [128, 1], f32, kind="ExternalInput")
    bin_d = nc.dram_tensor("bin", [128, 1], f32, kind="ExternalInput")
    bits_d = nc.dram_tensor("bits", [n_sites * 2, HB], f32, kind="ExternalOutput")

    with ExitStack() as ctx:
        tc = ctx.enter_context(tile.TileContext(nc))
        const = ctx.enter_context(tc.tile_pool(name="const", bufs=1))
        work = ctx.enter_context(tc.tile_pool(name="work", bufs=2))
        ps = ctx.enter_context(tc.tile_pool(name="ps", bufs=1, space="PSUM"))

        # Weights bounce through DVE copies: consumers then wait on the DVE
        # tick only, and the copy performs the f32 -> f32r rounding the
        # matmult input contract requires.
        cooked = {}
        for nm, dr, sh in (
            ("wz", wz_d, [128, 128]), ("wr", wr_d, [128, 128]),
            ("wn", wn_d, [128, 128]), ("qz", qz_d, [2, 128]),
            ("qr", qr_d, [2, 128]), ("qn", qn_d, [2, 128]),
            ("whd", whd_d, [128, 2]), ("bhn", bhn_d, [128, 1]),
            ("bz", bz_d, [128, 1]), ("br", br_d, [128, 1]),
            ("bin", bin_d, [128, 1]),
        ):
            raw = const.tile(sh, f32, name=f"{nm}_raw")
            dst = const.tile(sh, f32 if nm in ("bhn", "bz", "br", "bin") else f32r, name=f"{nm}_c")
            nc.sync.dma_start(raw[:], dr[:])
            nc.vector.tensor_copy(dst[:], raw[:])
            cooked[nm] = dst
        wz, wr, wn = cooked["wz"], cooked["wr"], cooked["wn"]
        qz, qr, qn = cooked["qz"], cooked["qr"], cooked["qn"]
        whd, bhn = cooked["whd"], cooked["bhn"]
        bz2, br2, bin2 = cooked["bz"], cooked["br"], cooked["bin"]

        uhat = const.tile([128, J, 2, HB], f32)
        nc.sync.dma_start(uhat[:], uhat_d[:])

        # Per-stream ping-pong state (streams are fully decoupled so the
        # scheduler can phase-shift them): hh rows 0-63 = h half A,
        # 64-127 = h half B; bb rows 0/1 = bit A/B.
        hhs = [
            [const.tile([128, CH], f32r, tag=f"hh{i}s{s}", name=f"hh{i}s{s}")
             for i in range(2)]
            for s in range(NS)
        ]
        bbs = [
            [const.tile([2, CH], f32r, tag=f"bb{i}s{s}", name=f"bb{i}s{s}")
             for i in range(2)]
            for s in range(NS)
        ]
        one128 = const.tile([128, 1], f32, name="one128")
        nc.vector.memset(one128[:], 1.0)
        for s in range(NS):
            for i in range(2):
                nc.vector.memzero(hhs[s][i][:])
                nc.vector.memzero(bbs[s][i][:])

        CSL = [slice(s * CH, (s + 1) * CH) for s in range(NS)]
        nt_prev = [None] * NS
        for t in range(n_sites):
            for s in range(NS):
                cs = CSL[s]
                cur = hhs[s][t % 2]
                nxt = hhs[s][(t + 1) % 2]
                bc = bbs[s][t % 2]
                bn = bbs[s][(t + 1) % 2]

                uh = work.tile([2, CH], f32, tag=f"uh{s}", name=f"uh{s}")
                nc.sync.dma_start(uh[:], uhat[t % 128 : t % 128 + 1, t // 128, :, cs])

                if nt_prev[s] is not None:
                    # Dummy 4-byte LDWEIGHTS reading last site's tanh output:
                    # advances PE's observed ACT tick past old activations,
                    # eliding psum-WAR semaphores on the matmuls below.
                    nc.tensor.ldweights(weights=nt_prev[s][0:1, 0:2].bitcast(bf16))

                pz = ps.tile([128, CH], f32, tag=f"z{s}", name=f"pz{s}")
                pr = ps.tile([128, CH], f32, tag=f"r{s}", name=f"pr{s}")
                pn = ps.tile([128, CH], f32, tag=f"n{s}", name=f"pn{s}")
                pq = ps.tile([128, CH], f32, tag=f"q{s}", name=f"pq{s}")
                # Q matmul first: needs only last site's bits, so bit*w_ihn
                # sits ready in PSUM before the gate chain arrives.
                nc.tensor.matmul(pq[:], qn[:], bc[:], start=True, stop=True)
                nc.tensor.matmul(pr[:], wr[:], cur[:], start=True, stop=False)
                nc.tensor.matmul(pr[:], qr[:], bc[:], start=False, stop=True)
                nc.tensor.matmul(pz[:], wz[:], cur[:], start=True, stop=False)
                nc.tensor.matmul(pz[:], qz[:], bc[:], start=False, stop=True)
                nc.tensor.matmul(pn[:], wn[:], cur[:], start=True, stop=True)

                rt = work.tile([128, CH], f32, tag=f"rt{s}", name=f"rt{s}")
                zt = work.tile([128, CH], f32, tag=f"zt{s}", name=f"zt{s}")
                nc.scalar.activation(rt[:], pr[:], AF.Sigmoid, bias=br2[:])
                nc.scalar.activation(zt[:], pz[:], AF.Sigmoid, bias=bz2[:])

                # rg = (ghn + b_hn) * r, then npre = rg + bit*w_ihn: both on
                # DVE back-to-back, no PE hop inside the chain (b_in rides
                # the tanh bias port).
                rg = work.tile([128, CH], f32, tag=f"rg{s}", name=f"rg{s}")
                npre = work.tile([128, CH], f32, tag=f"np{s}", name=f"np{s}")
                nc.vector.scalar_tensor_tensor(
                    rg[:], pn[:], bhn[:], rt[:], OP.add, OP.mult
                )
                nc.vector.tensor_tensor(npre[:], rg[:], pq[:], OP.add)

                nt = work.tile([128, CH], f32, tag=f"nt{s}", name=f"nt{s}")
                nc.scalar.activation(nt[:], npre[:], AF.Tanh, bias=bin2[:])

                # h' = (1-z)*n + z*h. zh = z*h computes early on GpSimd (off
                # the critical path); after tanh one STT gives
                # negyn = (z-1)*n and the f32r subtract lands h'.
                zh = work.tile([128, CH], f32, tag=f"zh{s}", name=f"zh{s}")
                negyn = work.tile([128, CH], f32, tag=f"ng{s}", name=f"ng{s}")
                nc.gpsimd.tensor_tensor(
                    zh[:], zt[:], cur[:].bitcast(f32), OP.mult
                )
                nc.vector.scalar_tensor_tensor(
                    negyn[:], zt[:], one128[:], nt[:], OP.subtract, OP.mult
                )
                nc.vector.tensor_tensor(nxt[:], zh[:], negyn[:], OP.subtract)

                # Head: ph = [logit_A; logit_B] into the q psum bank (tag
                # reuse), then bits = (uhat < logit) into the next bits tile.
                ph = ps.tile([2, CH], f32, tag=f"q{s}", name=f"ph{s}")
                nc.tensor.matmul(ph[:], whd[:], nxt[:], start=True, stop=True)
                nc.vector.tensor_tensor(bn[:], uh[:], ph[:], OP.is_lt)

                nc.sync.dma_start(bits_d[2 * t : 2 * t + 2, cs], bn[:].bitcast(f32))
                nt_prev[s] = nt

    if compile:
        nc.compile()
    return nc


def _pack_inputs(u, w_ih, w_hh, b_ih, b_hh, head_w, head_b):
    H = HIDDEN
    w_ih = np.asarray(w_ih, np.float32)
    w_hh = np.asarray(w_hh, np.float32)
    b_ih = np.asarray(b_ih, np.float32)
    b_hh = np.asarray(b_hh, np.float32)
    head_w = np.asarray(head_w, np.float32)
    head_b = np.asarray(head_b, np.float32)

    gates = {"r": slice(0, H), "z": slice(H, 2 * H), "n": slice(2 * H, 3 * H)}

    def bd(g):
        W = w_hh[gates[g], :].T.astype(np.float32)  # [64 in, 64 out]
        out = np.zeros((128, 128), np.float32)
        out[0:64, 0:64] = W
        out[64:128, 64:128] = W
        return out

    def qmat(g):
        wv = w_ih[gates[g], 0]
        out = np.zeros((2, 128), np.float32)
        out[0, 0:64] = wv
        out[1, 64:128] = wv
        return out

    def packed_bias(v):
        out = np.zeros((128, 1), np.float32)
        out[0:64, 0] = v
        out[64:128, 0] = v
        return out

    wz = bd("z")
    wr = bd("r")
    wn = bd("n")
    qz = qmat("z")
    qr = qmat("r")
    qn = qmat("n")
    bz = packed_bias(b_ih[gates["z"]] + b_hh[gates["z"]])
    br = packed_bias(b_ih[gates["r"]] + b_hh[gates["r"]])
    bin_ = packed_bias(b_ih[gates["n"]])

    bhn = packed_bias(b_hh[gates["n"]])

    whd = np.zeros((128, 2), np.float32)
    whd[0:64, 0] = head_w[0]
    whd[64:128, 1] = head_w[0]

    # Uhat in the [128, J, 2, HB] device layout: site t at partition t%128,
    # block t//128; last dims = (half, sample-within-half).
    u64 = np.asarray(u, np.float64)
    L = (np.log(u64) - np.log1p(-u64) - float(head_b[0])).astype(np.float32)  # [B, S]
    n_sites = u.shape[1]
    J = max(1, (n_sites + 127) // 128)
    uhats = []
    for c in range(N_CORES):
        Lc = L[c * B_LOCAL : (c + 1) * B_LOCAL].T  # [S, 2048]
        if n_sites < J * 128:
            Lc = np.pad(Lc, ((0, J * 128 - n_sites), (0, 0)))
        # site t -> [t % 128, t // 128]; halves on the next axis
        Lr = Lc.reshape(J, 128, 2, HB).transpose(1, 0, 2, 3)
        uhats.append(np.ascontiguousarray(Lr))
    return wz, wr, wn, qz, qr, qn, whd, bhn, bz, br, bin_, uhats


def kernel(u, w_ih, w_hh, b_ih, b_hh, head_w, head_b):
    from concourse.bass_utils import run_bass_kernel_spmd

    u = np.asarray(u)
    n_sites = u.shape[1]
    if n_sites not in _BUILD_CACHE:
        _BUILD_CACHE[n_sites] = _build(n_sites)
    nc = _BUILD_CACHE[n_sites]

    wz, wr, wn, qz, qr, qn, whd, bhn, bz, br, bin_, uhats = _pack_inputs(
        u, w_ih, w_hh, b_ih, b_hh, head_w, head_b
    )
    in_maps = [
        {
            "uhat": uhats[c], "wz": wz, "wr": wr, "wn": wn,
            "qz": qz, "qr": qr, "qn": qn, "whd": whd, "bhn": bhn,
            "bz": bz, "br": br, "bin": bin_,
        }
        for c in range(N_CORES)
    ]
    res = run_bass_kernel_spmd(nc, in_maps, list(range(N_CORES)))
    global LAST_RESULTS
    LAST_RESULTS = res
    out = np.empty((BATCH, n_sites), np.int32)
    for c in range(N_CORES):
        bits = res.results[c]["bits"].reshape(n_sites, 2 * HB)  # [S, 2048]
        out[c * B_LOCAL : (c + 1) * B_LOCAL] = bits.T.astype(np.int32)
    return out


# revision 18
# speedup vs baseline: 1.0348x; 1.0348x over previous
"""Trainium2 Bass kernel for autoregressive GRU sampling.

Problem: B=16384 samples, 1024 sequential sites, hidden=64, PyTorch GRU-cell
math with gates [r,z,n], Bernoulli sampling via pre-drawn uniforms.

Strategy:
  - Pure data parallel over 8 cores (2048 samples/core); the 1024-site scan
    is local to each core.
  - Packed layout: the core's 2048 samples split into halves A/B; partitions
    carry hidden(64) x half(2), free dim carries 1024 samples. Block-diagonal
    stationaries [128,128] contract both halves in one matmul group, so every
    elementwise op runs [128, 1024] instead of [64, 2048] (DVE/ACT cost is
    proportional to free size, halving the elementwise chain).
  - Gate biases ride the matmuls: the bits tile has a const-1.0 row, and the
    Q* stationaries carry [w_ih | bias] rows, so pre-activations arrive
    complete in PSUM. b_hn rides the rg STT scalar port; b_in rides qn.
  - The n-gate pre-activation is assembled IN PSUM: DVE writes
    rg = (ghn + b_hn) * r into a psum bank, then a K=3 matmul accumulates
    bit*w_ihn + b_in on top (start=False), so tanh reads the finished value
    and one DVE pass disappears.
  - fp32r matmuls (1 cycle/row vs fp32's 4): all matmul inputs are produced
    as float32r (DVE-rounded) per the BIR verifier's contract.
  - Host precomputes Uhat = logit(u) - head_b; the device compares
    Uhat < head_w.h' in logit space (is_lt), removing the head sigmoid.
  - Two 512-wide streams per site keep PE/ACT/DVE/Pool pipelined; d and zd
    run on GpSimd (SBUF-only ops) to offload DVE.
  - A 4-byte dummy LDWEIGHTS reading tanh output advances PE's observed-ACT
    tick so psum-WAR waits stay within Matmult's 2-sync-wait budget.
"""

import numpy as np
from contextlib import ExitStack

HIDDEN = 64
N_SITES = 1024
BATCH = 16384
N_CORES = 8
B_LOCAL = BATCH // N_CORES  # 2048
HB = B_LOCAL // 2           # 1024 samples per half
CH = 512                    # moving-operand chunk (one stream)
NS = HB // CH               # 2 streams

_BUILD_CACHE = {}


def _build(n_sites: int, compile: bool = True):
    import concourse.bass as bass
    import concourse.bacc as bacc
    import concourse.tile as tile
    from concourse import mybir

    f32 = mybir.dt.float32
    f32r = mybir.dt.float32r
    bf16 = mybir.dt.bfloat16
    AF = mybir.ActivationFunctionType
    OP = mybir.AluOpType
    J = max(1, (n_sites + 127) // 128)

    nc = bacc.Bacc()
    uhat_d = nc.dram_tensor("uhat", [128, J, 2, HB], f32, kind="ExternalInput")
    wz_d = nc.dram_tensor("wz", [128, 128], f32, kind="ExternalInput")
    wr_d = nc.dram_tensor("wr", [128, 128], f32, kind="ExternalInput")
    wn_d = nc.dram_tensor("wn", [128, 128], f32, kind="ExternalInput")
    qz_d = nc.dram_tensor("qz", [2, 128], f32, kind="ExternalInput")
    qr_d = nc.dram_tensor("qr", [2, 128], f32, kind="ExternalInput")
    qn_d = nc.dram_tensor("qn", [2, 128], f32, kind="ExternalInput")
    whd_d = nc.dram_tensor("whd", [128, 2], f32, kind="ExternalInput")
    bhn_d = nc.dram_tensor("bhn", [128, 1], f32, kind="ExternalInput")
    bz_d = nc.dram_tensor("bz", [128, 1], f32, kind="ExternalInput")
    br_d = nc.dram_tensor("br", [128, 1], f32, kind="ExternalInput")
    bin_d = nc.dram_tensor("bin", [128, 1], f32, kind="ExternalInput")
    bits_d = nc.dram_tensor("bits", [n_sites * 2, HB], f32, kind="ExternalOutput")

    with ExitStack() as ctx:
        tc = ctx.enter_context(tile.TileContext(nc))
        const = ctx.enter_context(tc.tile_pool(name="const", bufs=1))
        work = ctx.enter_context(tc.tile_pool(name="work", bufs=2))
        ps = ctx.enter_context(tc.tile_pool(name="ps", bufs=1, space="PSUM"))

        # Weights bounce through DVE copies: consumers then wait on the DVE
        # tick only, and the copy performs the f32 -> f32r rounding the
        # matmult input contract requires.
        cooked = {}
        for nm, dr, sh in (
            ("wz", wz_d, [128, 128]), ("wr", wr_d, [128, 128]),
            ("wn", wn_d, [128, 128]), ("qz", qz_d, [2, 128]),
            ("qr", qr_d, [2, 128]), ("qn", qn_d, [2, 128]),
            ("whd", whd_d, [128, 2]), ("bhn", bhn_d, [128, 1]),
            ("bz", bz_d, [128, 1]), ("br", br_d, [128, 1]),
            ("bin", bin_d, [128, 1]),
        ):
            raw = const.tile(sh, f32, name=f"{nm}_raw")
            dst = const.tile(sh, f32 if nm in ("bhn", "bz", "br", "bin") else f32r, name=f"{nm}_c")
            nc.sync.dma_start(raw[:], dr[:])
            nc.vector.tensor_copy(dst[:], raw[:])
            cooked[nm] = dst
        wz, wr, wn = cooked["wz"], cooked["wr"], cooked["wn"]
        qz, qr, qn = cooked["qz"], cooked["qr"], cooked["qn"]
        whd, bhn = cooked["whd"], cooked["bhn"]
        bz2, br2, bin2 = cooked["bz"], cooked["br"], cooked["bin"]

        uhat = const.tile([128, J, 2, HB], f32)
        nc.sync.dma_start(uhat[:], uhat_d[:])

        # Per-stream ping-pong state (streams are fully decoupled so the
        # scheduler can phase-shift them): hh rows 0-63 = h half A,
        # 64-127 = h half B; bb rows 0/1 = bit A/B.
        hhs = [
            [const.tile([128, CH], f32r, tag=f"hh{i}s{s}", name=f"hh{i}s{s}")
             for i in range(2)]
            for s in range(NS)
        ]
        bbs = [
            [const.tile([2, CH], f32r, tag=f"bb{i}s{s}", name=f"bb{i}s{s}")
             for i in range(2)]
            for s in range(NS)
        ]
        one128 = const.tile([128, 1], f32, name="one128")
        nc.vector.memset(one128[:], 1.0)
        for s in range(NS):
            for i in range(2):
                nc.vector.memzero(hhs[s][i][:])
                nc.vector.memzero(bbs[s][i][:])

        CSL = [slice(s * CH, (s + 1) * CH) for s in range(NS)]
        nt_prev = [None] * NS
        for t in range(n_sites):
            for s in range(NS):
                cs = CSL[s]
                cur = hhs[s][t % 2]
                nxt = hhs[s][(t + 1) % 2]
                bc = bbs[s][t % 2]
                bn = bbs[s][(t + 1) % 2]

                uh = work.tile([2, CH], f32, tag=f"uh{s}", name=f"uh{s}")
                nc.sync.dma_start(uh[:], uhat[t % 128 : t % 128 + 1, t // 128, :, cs])

                if nt_prev[s] is not None:
                    # Dummy 4-byte LDWEIGHTS reading last site's tanh output:
                    # advances PE's observed ACT tick past old activations,
                    # eliding psum-WAR semaphores on the matmuls below.
                    nc.tensor.ldweights(weights=nt_prev[s][0:1, 0:2].bitcast(bf16))

                pz = ps.tile([128, CH], f32, tag=f"z{s}", name=f"pz{s}")
                pr = ps.tile([128, CH], f32, tag=f"r{s}", name=f"pr{s}")
                pn = ps.tile([128, CH], f32, tag=f"n{s}", name=f"pn{s}")
                pq = ps.tile([128, CH], f32, tag=f"q{s}", name=f"pq{s}")
                nc.tensor.matmul(pr[:], wr[:], cur[:], start=True, stop=False)
                nc.tensor.matmul(pr[:], qr[:], bc[:], start=False, stop=True)
                nc.tensor.matmul(pz[:], wz[:], cur[:], start=True, stop=False)
                nc.tensor.matmul(pz[:], qz[:], bc[:], start=False, stop=True)
                nc.tensor.matmul(pn[:], wn[:], cur[:], start=True, stop=True)

                rt = work.tile([128, CH], f32, tag=f"rt{s}", name=f"rt{s}")
                zt = work.tile([128, CH], f32, tag=f"zt{s}", name=f"zt{s}")
                nc.scalar.activation(rt[:], pr[:], AF.Sigmoid, bias=br2[:])
                nc.scalar.activation(zt[:], pz[:], AF.Sigmoid, bias=bz2[:])

                # rg = (ghn + b_hn) * r -> q psum bank; qn matmul accumulates
                # bit*w_ihn on top -> npre sits finished in PSUM (b_in rides
                # the tanh bias port).
                nc.vector.scalar_tensor_tensor(
                    pq[:], pn[:], bhn[:], rt[:], OP.add, OP.mult
                )
                nc.tensor.matmul(
                    pq[:], qn[:], bc[:],
                    start=False, stop=True, skip_group_check=True,
                )

                nt = work.tile([128, CH], f32, tag=f"nt{s}", name=f"nt{s}")
                nc.scalar.activation(nt[:], pq[:], AF.Tanh, bias=bin2[:])

                # h' = (1-z)*n + z*h. zh = z*h computes early on GpSimd (off
                # the critical path); after tanh one STT gives
                # negyn = (z-1)*n and the f32r subtract lands h'.
                zh = work.tile([128, CH], f32, tag=f"zh{s}", name=f"zh{s}")
                negyn = work.tile([128, CH], f32, tag=f"ng{s}", name=f"ng{s}")
                nc.gpsimd.tensor_tensor(
                    zh[:], zt[:], cur[:].bitcast(f32), OP.mult
                )
                nc.vector.scalar_tensor_tensor(
                    negyn[:], zt[:], one128[:], nt[:], OP.subtract, OP.mult
                )
                nc.vector.tensor_tensor(nxt[:], zh[:], negyn[:], OP.subtract)

                # Head: ph = [logit_A; logit_B] into the q psum bank (tag
                # reuse), then bits = (uhat < logit) into the next bits tile.
                ph = ps.tile([2, CH], f32, tag=f"q{s}", name=f"ph{s}")
                nc.tensor.matmul(ph[:], whd[:], nxt[:], start=True, stop=True)
                nc.vector.tensor_tensor(bn[:], uh[:], ph[:], OP.is_lt)

                nc.sync.dma_start(bits_d[2 * t : 2 * t + 2, cs], bn[:].bitcast(f32))
                nt_prev[s] = nt

    if compile:
        nc.compile()
    return nc


def _pack_inputs(u, w_ih, w_hh, b_ih, b_hh, head_w, head_b):
    H = HIDDEN
    w_ih = np.asarray(w_ih, np.float32)
    w_hh = np.asarray(w_hh, np.float32)
    b_ih = np.asarray(b_ih, np.float32)
    b_hh = np.asarray(b_hh, np.float32)
    head_w = np.asarray(head_w, np.float32)
    head_b = np.asarray(head_b, np.float32)

    gates = {"r": slice(0, H), "z": slice(H, 2 * H), "n": slice(2 * H, 3 * H)}

    def bd(g):
        W = w_hh[gates[g], :].T.astype(np.float32)  # [64 in, 64 out]
        out = np.zeros((128, 128), np.float32)
        out[0:64, 0:64] = W
        out[64:128, 64:128] = W
        return out

    def qmat(g):
        wv = w_ih[gates[g], 0]
        out = np.zeros((2, 128), np.float32)
        out[0, 0:64] = wv
        out[1, 64:128] = wv
        return out

    def packed_bias(v):
        out = np.zeros((128, 1), np.float32)
        out[0:64, 0] = v
        out[64:128, 0] = v
        return out

    wz = bd("z")
    wr = bd("r")
    wn = bd("n")
    qz = qmat("z")
    qr = qmat("r")
    qn = qmat("n")
    bz = packed_bias(b_ih[gates["z"]] + b_hh[gates["z"]])
    br = packed_bias(b_ih[gates["r"]] + b_hh[gates["r"]])
    bin_ = packed_bias(b_ih[gates["n"]])

    bhn = packed_bias(b_hh[gates["n"]])

    whd = np.zeros((128, 2), np.float32)
    whd[0:64, 0] = head_w[0]
    whd[64:128, 1] = head_w[0]

    # Uhat in the [128, J, 2, HB] device layout: site t at partition t%128,
    # block t//128; last dims = (half, sample-within-half).
    u64 = np.asarray(u, np.float64)
    L = (np.log(u64) - np.log1p(-u64) - float(head_b[0])).astype(np.float32)  # [B, S]
    n_sites = u.shape[1]
    J = max(1, (n_sites + 127) // 128)
    uhats = []
    for c in range(N_CORES):
        Lc = L[c * B_LOCAL : (c + 1) * B_LOCAL].T  # [S, 2048]
        if n_sites < J * 128:
            Lc = np.pad(Lc, ((0, J * 128 - n_sites), (0, 0)))
        # site t -> [t % 128, t // 128]; halves on the next axis
        Lr = Lc.reshape(J, 128, 2, HB).transpose(1, 0, 2, 3)
        uhats.append(np.ascontiguousarray(Lr))
    return wz, wr, wn, qz, qr, qn, whd, bhn, bz, br, bin_, uhats


def kernel(u, w_ih, w_hh, b_ih, b_hh, head_w, head_b):
    from concourse.bass_utils import run_bass_kernel_spmd

    u = np.asarray(u)
    n_sites = u.shape[1]
    if n_sites not in _BUILD_CACHE:
        _BUILD_CACHE[n_sites] = _build(n_sites)
    nc = _BUILD_CACHE[n_sites]

    wz, wr, wn, qz, qr, qn, whd, bhn, bz, br, bin_, uhats = _pack_inputs(
        u, w_ih, w_hh, b_ih, b_hh, head_w, head_b
    )
    in_maps = [
        {
            "uhat": uhats[c], "wz": wz, "wr": wr, "wn": wn,
            "qz": qz, "qr": qr, "qn": qn, "whd": whd, "bhn": bhn,
            "bz": bz, "br": br, "bin": bin_,
        }
        for c in range(N_CORES)
    ]
    res = run_bass_kernel_spmd(nc, in_maps, list(range(N_CORES)))
    global LAST_RESULTS
    LAST_RESULTS = res
    out = np.empty((BATCH, n_sites), np.int32)
    for c in range(N_CORES):
        bits = res.results[c]["bits"].reshape(n_sites, 2 * HB)  # [S, 2048]
        out[c * B_LOCAL : (c + 1) * B_LOCAL] = bits.T.astype(np.int32)
    return out


# revision 19
# speedup vs baseline: 1.2412x; 1.1994x over previous
"""Trainium2 Bass kernel for autoregressive GRU sampling.

Problem: B=16384 samples, 1024 sequential sites, hidden=64, PyTorch GRU-cell
math with gates [r,z,n], Bernoulli sampling via pre-drawn uniforms.

Strategy:
  - Pure data parallel over 8 cores (2048 samples/core); the 1024-site scan
    is local to each core.
  - Packed layout: the core's 2048 samples split into halves A/B; partitions
    carry hidden(64) x half(2), free dim carries 1024 samples. Block-diagonal
    stationaries [128,128] contract both halves in one matmul group, so every
    elementwise op runs [128, 1024] instead of [64, 2048] (DVE/ACT cost is
    proportional to free size, halving the elementwise chain).
  - Gate biases ride the matmuls: the bits tile has a const-1.0 row, and the
    Q* stationaries carry [w_ih | bias] rows, so pre-activations arrive
    complete in PSUM. b_hn rides the rg STT scalar port; b_in rides qn.
  - The n-gate pre-activation is assembled IN PSUM: DVE writes
    rg = (ghn + b_hn) * r into a psum bank, then a K=3 matmul accumulates
    bit*w_ihn + b_in on top (start=False), so tanh reads the finished value
    and one DVE pass disappears.
  - fp32r matmuls (1 cycle/row vs fp32's 4): all matmul inputs are produced
    as float32r (DVE-rounded) per the BIR verifier's contract.
  - Host precomputes Uhat = logit(u) - head_b; the device compares
    Uhat < head_w.h' in logit space (is_lt), removing the head sigmoid.
  - Two 512-wide streams per site keep PE/ACT/DVE/Pool pipelined; d and zd
    run on GpSimd (SBUF-only ops) to offload DVE.
  - A 4-byte dummy LDWEIGHTS reading tanh output advances PE's observed-ACT
    tick so psum-WAR waits stay within Matmult's 2-sync-wait budget.
"""

import numpy as np
from contextlib import ExitStack

HIDDEN = 64
N_SITES = 1024
BATCH = 16384
N_CORES = 8
B_LOCAL = BATCH // N_CORES  # 2048
HB = B_LOCAL // 2           # 1024 samples per half
CH = 512                    # moving-operand chunk (one stream)
NS = HB // CH               # 2 streams

_BUILD_CACHE = {}


def _build(n_sites: int, compile: bool = True):
    import concourse.bass as bass
    import concourse.bacc as bacc
    import concourse.tile as tile
    from concourse import mybir

    f32 = mybir.dt.float32
    f32r = mybir.dt.float32r
    bf16 = mybir.dt.bfloat16
    AF = mybir.ActivationFunctionType
    OP = mybir.AluOpType
    J = max(1, (n_sites + 127) // 128)

    nc = bacc.Bacc()
    uhat_d = nc.dram_tensor("uhat", [128, J, 2, HB], f32, kind="ExternalInput")
    wz_d = nc.dram_tensor("wz", [128, 128], f32, kind="ExternalInput")
    wr_d = nc.dram_tensor("wr", [128, 128], f32, kind="ExternalInput")
    wn_d = nc.dram_tensor("wn", [128, 128], f32, kind="ExternalInput")
    qz_d = nc.dram_tensor("qz", [2, 128], f32, kind="ExternalInput")
    qr_d = nc.dram_tensor("qr", [2, 128], f32, kind="ExternalInput")
    qn_d = nc.dram_tensor("qn", [2, 128], f32, kind="ExternalInput")
    whd_d = nc.dram_tensor("whd", [128, 2], f32, kind="ExternalInput")
    bhn_d = nc.dram_tensor("bhn", [128, 1], f32, kind="ExternalInput")
    bz_d = nc.dram_tensor("bz", [128, 1], f32, kind="ExternalInput")
    br_d = nc.dram_tensor("br", [128, 1], f32, kind="ExternalInput")
    bin_d = nc.dram_tensor("bin", [128, 1], f32, kind="ExternalInput")
    bits_d = nc.dram_tensor("bits", [n_sites * 2, HB], f32, kind="ExternalOutput")

    with ExitStack() as ctx:
        tc = ctx.enter_context(tile.TileContext(nc))
        const = ctx.enter_context(tc.tile_pool(name="const", bufs=1))
        work = ctx.enter_context(tc.tile_pool(name="work", bufs=2))
        ps = ctx.enter_context(tc.tile_pool(name="ps", bufs=1, space="PSUM"))

        # Weights bounce through DVE copies: consumers then wait on the DVE
        # tick only, and the copy performs the f32 -> f32r rounding the
        # matmult input contract requires.
        cooked = {}
        for nm, dr, sh in (
            ("wz", wz_d, [128, 128]), ("wr", wr_d, [128, 128]),
            ("wn", wn_d, [128, 128]), ("qz", qz_d, [2, 128]),
            ("qr", qr_d, [2, 128]), ("qn", qn_d, [2, 128]),
            ("whd", whd_d, [128, 2]), ("bhn", bhn_d, [128, 1]),
            ("bz", bz_d, [128, 1]), ("br", br_d, [128, 1]),
            ("bin", bin_d, [128, 1]),
        ):
            raw = const.tile(sh, f32, name=f"{nm}_raw")
            dst = const.tile(sh, f32 if nm in ("bhn", "bz", "br", "bin") else f32r, name=f"{nm}_c")
            nc.sync.dma_start(raw[:], dr[:])
            nc.vector.tensor_copy(dst[:], raw[:])
            cooked[nm] = dst
        wz, wr, wn = cooked["wz"], cooked["wr"], cooked["wn"]
        qz, qr, qn = cooked["qz"], cooked["qr"], cooked["qn"]
        whd, bhn = cooked["whd"], cooked["bhn"]
        bz2, br2, bin2 = cooked["bz"], cooked["br"], cooked["bin"]

        uhat = const.tile([128, J, 2, HB], f32)
        nc.sync.dma_start(uhat[:], uhat_d[:])

        # Ping-pong state: hh rows 0-63 = h half A, 64-127 = h half B.
        # bb rows: 0 = bit A, 1 = bit B.
        hh = [const.tile([128, HB], f32r, tag=f"hh{i}", name=f"hh{i}") for i in range(2)]
        bb = [const.tile([2, HB], f32r, tag=f"bb{i}", name=f"bb{i}") for i in range(2)]
        one128 = const.tile([128, 1], f32, name="one128")
        nc.vector.memset(one128[:], 1.0)
        for ht in hh:
            nc.vector.memzero(ht[:])
        for bt in bb:
            nc.vector.memzero(bt[:])

        CSL = [slice(s * CH, (s + 1) * CH) for s in range(NS)]
        nt_prev = None
        for t in range(n_sites):
            cur = hh[t % 2]
            nxt = hh[(t + 1) % 2]
            bc = bb[t % 2]
            bn = bb[(t + 1) % 2]

            uh = work.tile([2, HB], f32, tag="uh")
            nc.sync.dma_start(uh[:], uhat[t % 128 : t % 128 + 1, t // 128, :, :])

            if nt_prev is not None:
                # Dummy 4-byte LDWEIGHTS reading last site's tanh output:
                # advances PE's observed ACT tick past site t-1's activations,
                # eliding psum-WAR semaphores on the matmuls below.
                nc.tensor.ldweights(weights=nt_prev[0:1, 0:2].bitcast(bf16))

            psZ, psR, psN, psQ = [], [], [], []
            for s in range(NS):
                cs = CSL[s]
                pz = ps.tile([128, CH], f32, tag=f"z{s}", name=f"pz{s}")
                pr = ps.tile([128, CH], f32, tag=f"r{s}", name=f"pr{s}")
                pn = ps.tile([128, CH], f32, tag=f"n{s}", name=f"pn{s}")
                nc.tensor.matmul(pr[:], wr[:], cur[:, cs], start=True, stop=False)
                nc.tensor.matmul(pr[:], qr[:], bc[:, cs], start=False, stop=True)
                nc.tensor.matmul(pz[:], wz[:], cur[:, cs], start=True, stop=False)
                nc.tensor.matmul(pz[:], qz[:], bc[:, cs], start=False, stop=True)
                nc.tensor.matmul(pn[:], wn[:], cur[:, cs], start=True, stop=True)
                psZ.append(pz)
                psR.append(pr)
                psN.append(pn)
                psQ.append(ps.tile([128, CH], f32, tag=f"q{s}", name=f"pq{s}"))

            rt = work.tile([128, HB], f32, tag="rt")
            zt = work.tile([128, HB], f32, tag="zt")
            for s in range(NS):
                nc.scalar.activation(rt[:, CSL[s]], psR[s][:], AF.Sigmoid, bias=br2[:])
            for s in range(NS):
                nc.scalar.activation(zt[:, CSL[s]], psZ[s][:], AF.Sigmoid, bias=bz2[:])

            # rg = (ghn + b_hn) * r -> q psum bank; qn matmul accumulates
            # bit*w_ihn on top -> npre sits finished in PSUM (b_in rides the
            # tanh bias port).
            for s in range(NS):
                nc.vector.scalar_tensor_tensor(
                    psQ[s][:], psN[s][:], bhn[:], rt[:, CSL[s]], OP.add, OP.mult
                )
            for s in range(NS):
                nc.tensor.matmul(
                    psQ[s][:], qn[:], bc[:, CSL[s]],
                    start=False, stop=True, skip_group_check=True,
                )

            nt = work.tile([128, HB], f32, tag="nt")
            for s in range(NS):
                nc.scalar.activation(nt[:, CSL[s]], psQ[s][:], AF.Tanh, bias=bin2[:])

            # h' = (1-z)*n + z*h. zh = z*h computes early on GpSimd (right
            # after sigmoid-z, off the critical path). After tanh, one STT
            # gives negyn = (z-1)*n and the f32r subtract lands h'.
            zh = work.tile([128, HB], f32, tag="zh")
            negyn = work.tile([128, HB], f32, tag="negyn")
            for s in range(NS):
                cs = CSL[s]
                nc.gpsimd.tensor_tensor(
                    zh[:, cs], zt[:, cs], cur[:, cs].bitcast(f32), OP.mult
                )
            for s in range(NS):
                cs = CSL[s]
                nc.vector.scalar_tensor_tensor(
                    negyn[:, cs], zt[:, cs], one128[:], nt[:, cs],
                    OP.subtract, OP.mult,
                )
                nc.vector.tensor_tensor(nxt[:, cs], zh[:, cs], negyn[:, cs], OP.subtract)

            # Head: ph = [logit_A; logit_B] into the q psum bank (tag reuse),
            # then bits = (uhat < logit) straight into the next bits tile.
            for s in range(NS):
                cs = CSL[s]
                ph = ps.tile([2, CH], f32, tag=f"q{s}", name=f"ph{s}")
                nc.tensor.matmul(ph[:], whd[:], nxt[:, cs], start=True, stop=True)
                nc.vector.tensor_tensor(bn[0:2, cs], uh[:, cs], ph[:], OP.is_lt)

            nc.sync.dma_start(bits_d[2 * t : 2 * t + 2, :], bn[0:2, :].bitcast(f32))
            nt_prev = nt

    if compile:
        nc.compile()
    return nc


def _pack_inputs(u, w_ih, w_hh, b_ih, b_hh, head_w, head_b):
    H = HIDDEN
    w_ih = np.asarray(w_ih, np.float32)
    w_hh = np.asarray(w_hh, np.float32)
    b_ih = np.asarray(b_ih, np.float32)
    b_hh = np.asarray(b_hh, np.float32)
    head_w = np.asarray(head_w, np.float32)
    head_b = np.asarray(head_b, np.float32)

    gates = {"r": slice(0, H), "z": slice(H, 2 * H), "n": slice(2 * H, 3 * H)}

    def bd(g):
        W = w_hh[gates[g], :].T.astype(np.float32)  # [64 in, 64 out]
        out = np.zeros((128, 128), np.float32)
        out[0:64, 0:64] = W
        out[64:128, 64:128] = W
        return out

    def qmat(g):
        wv = w_ih[gates[g], 0]
        out = np.zeros((2, 128), np.float32)
        out[0, 0:64] = wv
        out[1, 64:128] = wv
        return out

    def packed_bias(v):
        out = np.zeros((128, 1), np.float32)
        out[0:64, 0] = v
        out[64:128, 0] = v
        return out

    wz = bd("z")
    wr = bd("r")
    wn = bd("n")
    qz = qmat("z")
    qr = qmat("r")
    qn = qmat("n")
    bz = packed_bias(b_ih[gates["z"]] + b_hh[gates["z"]])
    br = packed_bias(b_ih[gates["r"]] + b_hh[gates["r"]])
    bin_ = packed_bias(b_ih[gates["n"]])

    bhn = packed_bias(b_hh[gates["n"]])

    whd = np.zeros((128, 2), np.float32)
    whd[0:64, 0] = head_w[0]
    whd[64:128, 1] = head_w[0]

    # Uhat in the [128, J, 2, HB] device layout: site t at partition t%128,
    # block t//128; last dims = (half, sample-within-half).
    u64 = np.asarray(u, np.float64)
    L = (np.log(u64) - np.log1p(-u64) - float(head_b[0])).astype(np.float32)  # [B, S]
    n_sites = u.shape[1]
    J = max(1, (n_sites + 127) // 128)
    uhats = []
    for c in range(N_CORES):
        Lc = L[c * B_LOCAL : (c + 1) * B_LOCAL].T  # [S, 2048]
        if n_sites < J * 128:
            Lc = np.pad(Lc, ((0, J * 128 - n_sites), (0, 0)))
        # site t -> [t % 128, t // 128]; halves on the next axis
        Lr = Lc.reshape(J, 128, 2, HB).transpose(1, 0, 2, 3)
        uhats.append(np.ascontiguousarray(Lr))
    return wz, wr, wn, qz, qr, qn, whd, bhn, bz, br, bin_, uhats


def kernel(u, w_ih, w_hh, b_ih, b_hh, head_w, head_b):
    from concourse.bass_utils import run_bass_kernel_spmd

    u = np.asarray(u)
    n_sites = u.shape[1]
    if n_sites not in _BUILD_CACHE:
        _BUILD_CACHE[n_sites] = _build(n_sites)
    nc = _BUILD_CACHE[n_sites]

    wz, wr, wn, qz, qr, qn, whd, bhn, bz, br, bin_, uhats = _pack_inputs(
        u, w_ih, w_hh, b_ih, b_hh, head_w, head_b
    )
    in_maps = [
        {
            "uhat": uhats[c], "wz": wz, "wr": wr, "wn": wn,
            "qz": qz, "qr": qr, "qn": qn, "whd": whd, "bhn": bhn,
            "bz": bz, "br": br, "bin": bin_,
        }
        for c in range(N_CORES)
    ]
    res = run_bass_kernel_spmd(nc, in_maps, list(range(N_CORES)))
    global LAST_RESULTS
    LAST_RESULTS = res
    out = np.empty((BATCH, n_sites), np.int32)
    for c in range(N_CORES):
        bits = res.results[c]["bits"].reshape(n_sites, 2 * HB)  # [S, 2048]
        out[c * B_LOCAL : (c + 1) * B_LOCAL] = bits.T.astype(np.int32)
    return out
